# revision 16
# baseline (speedup 1.0000x reference)
"""Trainium2 Bass kernel for nn_Encoder_57062935494680 (GAT-style GNN encoder).

Strategy (8 NeuronCores, node-sharded SPMD, single launch):
  - Nodes partitioned into 8 contiguous blocks of N/8. Weights replicated.
  - GAT edge softmax is reformulated densely: a host-built edge-count matrix
    C^T [src, dst] (bf16, exact small ints) turns gather/scatter segment ops
    into dense matmuls:  agg[d] = sum_s C[s,d]*exp(lrelu(a_s+a_d)) * h[s] / z[d].
    (Mathematically identical to the reference's segment softmax; the max
    subtraction is dropped since |logits| <= ~15 is safe in fp32.)
  - Two on-device AllGather sync points: (1) h + attention logit vectors,
    (2) h2 (natural, bf16) + h2^T (f32).
  - Batchnorm stats are computed redundantly on every core from the gathered
    h2^T (streaming reduction, no rank-dependent indexing); each core then
    decodes only its own node block from its local h2^T.
  - Readout (graph_neigh @ h2) uses a host-transposed bf16 slice of
    graph_neigh as the stationary operand.

kernel(**inputs) takes FULL inputs, returns the FULL 6-tuple
(h2, h3, ret, ret_a, h2, h2_a) matching reference.reference().
"""
import os
import sys

sys.path.insert(0, "/opt/trn_rl_repo")

import numpy as np
import ml_dtypes

BF16 = ml_dtypes.bfloat16

# ---------------------------------------------------------------- sizes
SMALL = bool(int(os.environ.get("BASS_GNN_SMALL", "0")))
NCORE = 8
if SMALL:
    N, IN, E = 1024, 384, 8192
else:
    N, IN, E = 8192, 3000, 262144
HID, OUT = 256, 64
EPS = 1e-5
NLOC = N // NCORE          # nodes per core
MS = NLOC // 128           # 128-row subtiles per core
ST = N // 128              # 128-row s-tiles over all nodes
WH = 528                   # hcat width (2*(HID+1) = 514, padded)
WH2 = 144                  # h2cat width (64+64+1 = 129, padded)
KT = [(k, min(128, IN - k)) for k in range(0, IN, 128)]   # IN k-tiles
NCH = [(c, min(500, IN - c)) for c in range(0, IN, 500)]  # h3 out chunks

_RUNNER = None


# ================================================================ device code
def _build_bass():
    import concourse.bass as bass
    import concourse.mybir as mybir
    import concourse.tile as tile
    from concourse import bacc
    from concourse.masks import make_identity

    dt = mybir.dt
    AF = mybir.ActivationFunctionType
    OP = mybir.AluOpType
    f32, bf16 = dt.float32, dt.bfloat16
    f32r = dt.float32r

    nc = bacc.Bacc(None, target_bir_lowering=False, debug=False)

    # ---------------- per-core external inputs
    featT0 = nc.declare_dram_parameter("featT0", [IN, NLOC], f32, isOutput=False)
    featT1 = nc.declare_dram_parameter("featT1", [IN, NLOC], f32, isOutput=False)
    ct_s = nc.declare_dram_parameter("ct_s", [N, NLOC], bf16, isOutput=False)
    gnT_s = nc.declare_dram_parameter("gnT_s", [N, NLOC], bf16, isOutput=False)
    W1 = nc.declare_dram_parameter("W1", [IN, HID], f32, isOutput=False)
    att = nc.declare_dram_parameter("att", [2 * HID], f32, isOutput=False)
    W2 = nc.declare_dram_parameter("W2", [HID, OUT], f32, isOutput=False)
    Wd1 = nc.declare_dram_parameter("Wd1", [OUT, HID], f32, isOutput=False)
    bd1 = nc.declare_dram_parameter("bd1", [HID], f32, isOutput=False)
    gamma = nc.declare_dram_parameter("gamma", [HID], f32, isOutput=False)
    beta = nc.declare_dram_parameter("beta", [HID], f32, isOutput=False)
    Wd2bf = nc.declare_dram_parameter("Wd2bf", [HID, IN], bf16, isOutput=False)
    bd2 = nc.declare_dram_parameter("bd2", [IN], f32, isOutput=False)
    discW = nc.declare_dram_parameter("discW", [OUT, OUT], f32, isOutput=False)
    discb = nc.declare_dram_parameter("discb", [1], f32, isOutput=False)

    # ---------------- per-core outputs
    out_h2 = nc.declare_dram_parameter("out_h2", [NLOC, OUT], f32, isOutput=True)
    out_h2a = nc.declare_dram_parameter("out_h2a", [NLOC, OUT], f32, isOutput=True)
    out_h3 = nc.declare_dram_parameter("out_h3", [NLOC, IN], f32, isOutput=True)
    out_ret = nc.declare_dram_parameter("out_ret", [NLOC, 2], f32, isOutput=True)
    out_reta = nc.declare_dram_parameter("out_reta", [NLOC, 2], f32, isOutput=True)

    # ---------------- internal DRAM (collective staging)
    hcat = nc.dram_tensor("hcat", [NLOC, WH], bf16)
    hfull = nc.dram_tensor("hfull", [N, WH], bf16, addr_space="Shared")
    acat = nc.dram_tensor("acat", [128, 2 * MS], f32)
    afull = nc.dram_tensor("afull", [NCORE * 128, 2 * MS], f32, addr_space="Shared")
    adst0 = nc.dram_tensor("adst0", [NLOC], f32)
    adst1 = nc.dram_tensor("adst1", [NLOC], f32)
    h2cat = nc.dram_tensor("h2cat", [NLOC, WH2], bf16)
    h2full = nc.dram_tensor("h2full", [N, WH2], bf16, addr_space="Shared")
    h2tp = nc.dram_tensor("h2tp", [128, NLOC], f32)
    h2tpfull = nc.dram_tensor("h2tpfull", [NCORE * 128, NLOC], f32, addr_space="Shared")

    RG = [list(range(NCORE))]

    with tile.TileContext(nc) as tc:
        with (
            tc.tile_pool(name="pconst", bufs=1) as pc,
            tc.tile_pool(name="ppersist", bufs=1) as pp,
            tc.tile_pool(name="pbig", bufs=3) as pb,
            tc.tile_pool(name="psml", bufs=3) as pm,
            tc.tile_pool(name="pP", bufs=3) as pPp,
            tc.tile_pool(name="pev", bufs=2) as pe,
            tc.tile_pool(name="ps", bufs=8, space="PSUM") as ps,
        ):
            def psum():
                return ps.tile([128, 512], f32, tag="ps", name="pst")

            # ======== setup: constants & broadcast helpers
            ident = pc.tile([128, 128], f32)
            make_identity(nc, ident)
            ones1p = pc.tile([1, 128], f32)
            nc.vector.memset(ones1p[:], 1.0)
            onescol_bf = pc.tile([128, 1], bf16)
            nc.vector.memset(onescol_bf[:], 1.0)

            att_row = pc.tile([1, 2 * HID], f32)
            nc.sync.dma_start(att_row[:], att[None, :])
            att_bc = pc.tile([128, 2 * HID], f32)
            pbb = psum()
            nc.tensor.matmul(pbb[:, :], ones1p[:], att_row[:, 0:512], start=True, stop=True)
            nc.vector.tensor_copy(att_bc[:, 0:512], pbb[:, :])

            bd2_row = pc.tile([1, IN], f32)
            nc.sync.dma_start(bd2_row[:], bd2[None, :])
            bd2_bc = pc.tile([128, IN], bf16)
            for c0, cw in NCH:
                pbb = psum()
                nc.tensor.matmul(pbb[:, :cw], ones1p[:], bd2_row[:, c0:c0 + cw], start=True, stop=True)
                nc.vector.tensor_copy(bd2_bc[:, c0:c0 + cw], pbb[:, :cw])

            db_row = pc.tile([1, 1], f32)
            nc.sync.dma_start(db_row[:], discb[None, :])
            db_bc = pc.tile([128, 1], f32)
            pbb = psum()
            nc.tensor.matmul(pbb[:, :1], ones1p[:], db_row[:], start=True, stop=True)
            nc.vector.tensor_copy(db_bc[:], pbb[:, :1])

            # small weights
            Wd1_sb = pc.tile([OUT, HID], f32r)
            nc.sync.dma_start(Wd1_sb[:], Wd1[:, :].bitcast(f32r))
            W2_sb = pc.tile([128, 2, OUT], f32)
            nc.sync.dma_start(W2_sb[:], W2.rearrange("(m p) o -> p m o", p=128))
            discW_sb = pc.tile([OUT, OUT], f32)
            nc.sync.dma_start(discW_sb[:], discW[:, :])
            bd1_sb = pc.tile([128, 2], f32)
            nc.sync.dma_start(bd1_sb[:], bd1.rearrange("(m p) -> p m", p=128))
            gamma_sb = pc.tile([128, 2], f32)
            nc.sync.dma_start(gamma_sb[:], gamma.rearrange("(m p) -> p m", p=128))
            beta_sb = pc.tile([128, 2], f32)
            nc.sync.dma_start(beta_sb[:], beta.rearrange("(m p) -> p m", p=128))

            # ======== phase A: h = feat @ W1 (per encode), logit vectors
            for e, (featT_e, adst_e) in enumerate(((featT0, adst0), (featT1, adst1))):
                ph = [psum() for _ in range(MS)]
                for ki, (k0, kw) in enumerate(KT):
                    ft = pb.tile([128, NLOC], f32r, tag="ft")
                    nc.sync.dma_start(ft[:kw, :], featT_e[k0:k0 + kw, :].bitcast(f32r))
                    wt = pm.tile([128, HID], f32r, tag="wt")
                    nc.sync.dma_start(wt[:kw, :], W1[k0:k0 + kw, :].bitcast(f32r))
                    for m in range(MS):
                        nc.tensor.matmul(
                            ph[m][:, :HID],
                            ft[:kw, m * 128:(m + 1) * 128],
                            wt[:kw, :],
                            start=(ki == 0), stop=(ki == len(KT) - 1),
                        )
                for m in range(MS):
                    h_sb = pe.tile([128, HID], f32, tag="h_sb")
                    nc.scalar.activation(h_sb[:], ph[m][:, :HID], AF.Copy)
                    hbf = pe.tile([128, HID], bf16, tag="hbf")
                    nc.vector.tensor_copy(hbf[:], ph[m][:, :HID])
                    nc.sync.dma_start(hcat[m * 128:(m + 1) * 128, e * 257:e * 257 + HID], hbf[:])
                    nc.sync.dma_start(hcat[m * 128:(m + 1) * 128, e * 257 + HID:e * 257 + HID + 1], onescol_bf[:])
                    # attention logits a_src/a_dst for this node block
                    tmp = pe.tile([128, HID], f32, tag="tmp")
                    nc.vector.tensor_mul(tmp[:], h_sb[:], att_bc[:, 0:HID])
                    asr = pe.tile([128, 1], f32, tag="asr")
                    nc.vector.reduce_sum(asr[:], tmp[:], axis=mybir.AxisListType.X)
                    nc.sync.dma_start(acat[:, e * MS + m:e * MS + m + 1], asr[:])
                    nc.vector.tensor_mul(tmp[:], h_sb[:], att_bc[:, HID:2 * HID])
                    ads = pe.tile([128, 1], f32, tag="ads")
                    nc.vector.reduce_sum(ads[:], tmp[:], axis=mybir.AxisListType.X)
                    nc.sync.dma_start(adst_e[m * 128:(m + 1) * 128][:, None], ads[:])

            # ======== AllGather #1
            nc.gpsimd.collective_compute(
                "AllGather", OP.bypass, ins=[hcat[:]], outs=[hfull[:]], replica_groups=RG)
            nc.gpsimd.collective_compute(
                "AllGather", OP.bypass, ins=[acat[:]], outs=[afull[:]], replica_groups=RG)

            # ======== phase B: dense edge-softmax aggregation
            # gather a_src columns (p-major) and a_dst broadcast rows
            asrc_sb = []
            adst_bc = []
            for e in range(2):
                a = pc.tile([128, ST], f32, tag=f"asrc{e}", name=f"asrc{e}")
                for r in range(NCORE):
                    nc.sync.dma_start(
                        a[:, r * MS:(r + 1) * MS],
                        afull[r * 128:(r + 1) * 128, e * MS:(e + 1) * MS])
                asrc_sb.append(a)
                row = pm.tile([1, NLOC], f32, tag="adrow")
                nc.sync.dma_start(row[:], (adst0 if e == 0 else adst1)[None, :])
                bc = pc.tile([128, NLOC], f32, tag=f"adbc{e}", name=f"adbc{e}")
                for c in range(0, NLOC, 512):
                    cw = min(512, NLOC - c)
                    pbb = psum()
                    nc.tensor.matmul(pbb[:, :cw], ones1p[:], row[:, c:c + cw], start=True, stop=True)
                    nc.vector.tensor_copy(bc[:, c:c + cw], pbb[:, :cw])
                adst_bc.append(bc)

            h1T_sb = [pp.tile([128, 2, NLOC], f32, tag=f"h1T{e}", name=f"h1T{e}") for e in range(2)]
            for e in range(2):
                pg = [psum() for _ in range(MS)]
                for st in range(ST):
                    ct = pb.tile([128, NLOC], bf16, tag="ct")
                    nc.sync.dma_start(ct[:], ct_s[st * 128:(st + 1) * 128, :])
                    rhs = pm.tile([128, 257], bf16, tag="rhs")
                    nc.sync.dma_start(rhs[:], hfull[st * 128:(st + 1) * 128, e * 257:(e + 1) * 257])
                    Et = pPp.tile([128, NLOC], f32, tag="Et")
                    nc.scalar.activation(Et[:], adst_bc[e][:], AF.Prelu,
                                         bias=asrc_sb[e][:, st:st + 1], alpha=0.2)
                    Xt = pPp.tile([128, NLOC], bf16, tag="Xt")
                    nc.scalar.activation(Xt[:], Et[:], AF.Exp)
                    Pt = pPp.tile([128, NLOC], bf16, tag="Pt")
                    nc.vector.tensor_mul(Pt[:], Xt[:], ct[:])
                    for m in range(MS):
                        nc.tensor.matmul(
                            pg[m][:, :257],
                            Pt[:, m * 128:(m + 1) * 128],
                            rhs[:],
                            start=(st == 0), stop=(st == ST - 1),
                        )
                # evict: alpha-normalize, elu, transpose -> h1T
                for m in range(MS):
                    zeps = pe.tile([128, 1], f32, tag="zeps")
                    nc.vector.tensor_scalar_add(zeps[:], pg[m][:, 256:257], 1e-16)
                    rz = pe.tile([128, 1], f32, tag="rz")
                    nc.vector.reciprocal(rz[:], zeps[:])
                    nm = pe.tile([128, HID], f32, tag="nm")
                    nc.vector.tensor_scalar(nm[:], pg[m][:, 0:HID], rz[:], None, OP.mult)
                    emn = pe.tile([128, HID], f32, tag="emn")
                    nc.vector.tensor_scalar_min(emn[:], nm[:], 0.0)
                    nc.scalar.activation(emn[:], emn[:], AF.Exp)
                    epo = pe.tile([128, HID], f32, tag="epo")
                    nc.vector.tensor_scalar(epo[:], nm[:], 0.0, -1.0, OP.max, OP.add)
                    h1 = pe.tile([128, HID], f32, tag="h1")
                    nc.vector.tensor_add(h1[:], emn[:], epo[:])
                    for kt2 in range(2):
                        pt = psum()
                        nc.tensor.transpose(pt[:, :128], h1[:, kt2 * 128:(kt2 + 1) * 128], ident[:])
                        nc.vector.tensor_copy(h1T_sb[e][:, kt2, m * 128:(m + 1) * 128], pt[:, :128])

            # ======== phase C: h2 = h1 @ W2 (+ transposes, AG#2 staging)
            h2T_sb = [pp.tile([OUT, NLOC], f32, tag=f"h2T{e}", name=f"h2T{e}") for e in range(2)]
            for e in range(2):
                for m in range(MS):
                    p2 = psum()
                    for kt2 in range(2):
                        nc.tensor.matmul(
                            p2[:, :OUT],
                            h1T_sb[e][:, kt2, m * 128:(m + 1) * 128],
                            W2_sb[:, kt2, :],
                            start=(kt2 == 0), stop=(kt2 == 1),
                        )
                    h2s = pe.tile([128, OUT], f32, tag="h2s")
                    nc.scalar.activation(h2s[:], p2[:, :OUT], AF.Copy)
                    nc.sync.dma_start(
                        (out_h2 if e == 0 else out_h2a)[m * 128:(m + 1) * 128, :], h2s[:])
                    h2b = pe.tile([128, OUT], bf16, tag="h2b")
                    nc.vector.tensor_copy(h2b[:], p2[:, :OUT])
                    nc.sync.dma_start(h2cat[m * 128:(m + 1) * 128, e * OUT:(e + 1) * OUT], h2b[:])
                    if e == 0:
                        nc.sync.dma_start(h2cat[m * 128:(m + 1) * 128, 128:129], onescol_bf[:])
                    ptr = psum()
                    nc.tensor.transpose(ptr[:OUT, :128], h2s[:, :], ident[:])
                    nc.vector.tensor_copy(h2T_sb[e][:, m * 128:(m + 1) * 128], ptr[:OUT, :128])
                nc.sync.dma_start(h2tp[e * OUT:(e + 1) * OUT, :], h2T_sb[e][:])

            # ======== AllGather #2
            nc.gpsimd.collective_compute(
                "AllGather", OP.bypass, ins=[h2cat[:]], outs=[h2full[:]], replica_groups=RG)
            nc.gpsimd.collective_compute(
                "AllGather", OP.bypass, ins=[h2tp[:]], outs=[h2tpfull[:]], replica_groups=RG)

            # ======== phase D: decoder (BN stats over all nodes, decode own block)
            NCHZ = [(c, min(512, NLOC - c)) for c in range(0, NLOC, 512)]
            nzch = len(NCHZ) * NCORE
            acc_s = pc.tile([128, 2, nzch], f32)
            acc_q = pc.tile([128, 2, nzch], f32)
            for r in range(NCORE):
                for ci, (c0, cw) in enumerate(NCHZ):
                    rz_t = pm.tile([OUT, 512], f32r, tag="rz_t")
                    nc.sync.dma_start(rz_t[:, :cw], h2tpfull[r * 128:r * 128 + OUT, c0:c0 + cw].bitcast(f32r))
                    for m2 in range(2):
                        pz = psum()
                        nc.tensor.matmul(
                            pz[:, :cw],
                            Wd1_sb[:, m2 * 128:(m2 + 1) * 128],
                            rz_t[:, :cw],
                            start=True, stop=True)
                        idx = r * len(NCHZ) + ci
                        nc.vector.reduce_sum(acc_s[:, m2, idx:idx + 1], pz[:, :cw], axis=mybir.AxisListType.X)
                        sqs = pe.tile([128, 512], f32, tag="h3s")
                        nc.scalar.activation(sqs[:, :cw], pz[:, :cw], AF.Square,
                                             accum_out=acc_q[:, m2, idx:idx + 1])
            # stats -> scale/shift
            ssum = pc.tile([128, 2], f32)
            qsum = pc.tile([128, 2], f32)
            for m2 in range(2):
                nc.vector.reduce_sum(ssum[:, m2:m2 + 1], acc_s[:, m2, :], axis=mybir.AxisListType.X)
                nc.vector.reduce_sum(qsum[:, m2:m2 + 1], acc_q[:, m2, :], axis=mybir.AxisListType.X)
            mur = pc.tile([128, 2], f32)
            nc.vector.tensor_scalar_mul(mur[:], ssum[:], 1.0 / N)
            var = pc.tile([128, 2], f32)
            msq = pe.tile([128, 2], f32, tag="msq")
            nc.vector.tensor_mul(msq[:], mur[:], mur[:])
            nc.vector.tensor_scalar_mul(var[:], qsum[:], 1.0 / N)
            nc.vector.tensor_sub(var[:], var[:], msq[:])
            eps_col = pc.tile([128, 1], f32)
            nc.vector.memset(eps_col[:], EPS)
            std = pc.tile([128, 2], f32)
            nc.scalar.activation(std[:], var[:], AF.Sqrt, bias=eps_col[:])
            rs = pc.tile([128, 2], f32)
            nc.vector.reciprocal(rs[:], std[:])
            scl = pc.tile([128, 2], f32)
            nc.vector.tensor_mul(scl[:], rs[:], gamma_sb[:])
            # shift = (bd1 - mu_raw) * scl + beta
            sh = pc.tile([128, 2], f32)
            nc.vector.tensor_sub(sh[:], bd1_sb[:], mur[:])
            nc.vector.tensor_mul(sh[:], sh[:], scl[:])
            nc.vector.tensor_add(sh[:], sh[:], beta_sb[:])
            # own-block z, normalize, elu
            znT = pp.tile([128, 2, NLOC], f32)
            for m2 in range(2):
                for c0, cw in NCHZ:
                    pz = psum()
                    nc.tensor.matmul(
                        pz[:, :cw],
                        Wd1_sb[:, m2 * 128:(m2 + 1) * 128].bitcast(f32),
                        h2T_sb[0][:, c0:c0 + cw],
                        start=True, stop=True)
                    nc.vector.tensor_scalar(
                        znT[:, m2, c0:c0 + cw], pz[:, :cw],
                        scl[:, m2:m2 + 1], sh[:, m2:m2 + 1], OP.mult, OP.add)
            zel = pp.tile([128, 2, NLOC], bf16)
            tmp1 = pp.tile([128, 2, NLOC], f32)
            nc.vector.tensor_scalar_min(tmp1[:], znT[:], 0.0)
            nc.scalar.activation(tmp1[:], tmp1[:], AF.Exp)
            nc.vector.tensor_scalar(zel[:], znT[:], 0.0, -1.0, OP.max, OP.add)
            nc.vector.tensor_add(zel[:], zel[:], tmp1[:])
            # h3 = zel^T @ Wd2 + bd2
            Wd2r = Wd2bf.rearrange("(m p) i -> p m i", p=128)
            for c0, cw in NCH:
                wd2t = pm.tile([128, 2, 500], bf16, tag="wd2t")
                nc.sync.dma_start(wd2t[:, :, :cw], Wd2r[:, :, c0:c0 + cw])
                p3 = [psum() for _ in range(MS)]
                for m in range(MS):
                    for kt2 in range(2):
                        nc.tensor.matmul(
                            p3[m][:, :cw],
                            zel[:, kt2, m * 128:(m + 1) * 128],
                            wd2t[:, kt2, :cw],
                            start=(kt2 == 0), stop=(kt2 == 1))
                    h3s = pe.tile([128, 512], f32, tag="h3s")
                    nc.vector.tensor_add(h3s[:, :cw], p3[m][:, :cw], bd2_bc[:, c0:c0 + cw])
                    nc.sync.dma_start(out_h3[m * 128:(m + 1) * 128, c0:c0 + cw], h3s[:, :cw])

            # ======== phase E: readout + discriminator
            pr = [psum() for _ in range(MS)]
            for st in range(ST):
                gt = pb.tile([128, NLOC], bf16, tag="gt")
                nc.sync.dma_start(gt[:], gnT_s[st * 128:(st + 1) * 128, :])
                rr = pm.tile([128, WH2], bf16, tag="rr")
                nc.sync.dma_start(rr[:], h2full[st * 128:(st + 1) * 128, :])
                for m in range(MS):
                    nc.tensor.matmul(
                        pr[m][:, :WH2],
                        gt[:, m * 128:(m + 1) * 128],
                        rr[:],
                        start=(st == 0), stop=(st == ST - 1))
            for m in range(MS):
                rm = pe.tile([128, 1], f32, tag="rm")
                nc.vector.reciprocal(rm[:], pr[m][:, 128:129])
                gs = []
                for gi in range(2):
                    g = pe.tile([128, OUT], f32, tag=f"g{gi}", name=f"g{gi}")
                    nc.vector.tensor_scalar(g[:], pr[m][:, gi * OUT:(gi + 1) * OUT], rm[:], None, OP.mult)
                    sq = pe.tile([128, OUT], f32, tag="sq")
                    nc.vector.tensor_mul(sq[:], g[:], g[:])
                    ssn = pe.tile([128, 1], f32, tag="ssn")
                    nc.vector.reduce_sum(ssn[:], sq[:], axis=mybir.AxisListType.X)
                    nc.scalar.activation(ssn[:], ssn[:], AF.Sqrt)
                    nc.vector.tensor_scalar_max(ssn[:], ssn[:], 1e-12)
                    nc.vector.reciprocal(ssn[:], ssn[:])
                    nc.vector.tensor_scalar(g[:], g[:], ssn[:], None, OP.mult)
                    nc.scalar.activation(g[:], g[:], AF.Sigmoid)
                    gs.append(g)
                # t1 = h2 @ discW, t2 = h2a @ discW  (own block)
                ts_ = []
                for e in range(2):
                    pd = psum()
                    nc.tensor.matmul(
                        pd[:, :OUT],
                        h2T_sb[e][:, m * 128:(m + 1) * 128],
                        discW_sb[:],
                        start=True, stop=True)
                    tt = pe.tile([128, OUT], f32, tag=f"tt{e}", name=f"tt{e}")
                    nc.vector.tensor_copy(tt[:], pd[:, :OUT])
                    ts_.append(tt)
                prod = pe.tile([128, OUT], f32, tag="prod")
                retv = pe.tile([128, 2], f32, tag="retv")
                retav = pe.tile([128, 2], f32, tag="retav")
                for col, (tv, gv, dest) in enumerate(
                        [(ts_[0], gs[0], (retv, 0)), (ts_[1], gs[0], (retv, 1)),
                         (ts_[1], gs[1], (retav, 0)), (ts_[0], gs[1], (retav, 1))]):
                    nc.vector.tensor_mul(prod[:], tv[:], gv[:])
                    dtile, dcol = dest
                    nc.vector.reduce_sum(dtile[:, dcol:dcol + 1], prod[:], axis=mybir.AxisListType.X)
                nc.vector.tensor_scalar_add(retv[:], retv[:], db_bc[:])
                nc.vector.tensor_scalar_add(retav[:], retav[:], db_bc[:])
                nc.sync.dma_start(out_ret[m * 128:(m + 1) * 128, :], retv[:])
                nc.sync.dma_start(out_reta[m * 128:(m + 1) * 128, :], retav[:])

    nc.compile()
    return nc


# Need mybir at module level for AxisListType inside _build_bass
import concourse.mybir as mybir  # noqa: E402


# ================================================================ host runner
class _SpmdRunner:
    def __init__(self, nc, n_cores):
        import jax
        from jax.sharding import Mesh, PartitionSpec, NamedSharding
        from jax.experimental.shard_map import shard_map
        from concourse.bass2jax import (
            _bass_exec_p, install_neuronx_cc_hook, partition_id_tensor)

        install_neuronx_cc_hook()
        self.jax = jax
        self.nc = nc
        self.n_cores = n_cores
        partition_name = nc.partition_id_tensor.name if nc.partition_id_tensor else None
        in_names, out_names, out_avals, zero_shapes = [], [], [], []
        for alloc in nc.m.functions[0].allocations:
            if not isinstance(alloc, mybir.MemoryLocationSet):
                continue
            name = alloc.memorylocations[0].name
            if alloc.kind == "ExternalInput":
                if name != partition_name:
                    in_names.append(name)
            elif alloc.kind == "ExternalOutput":
                shape = tuple(alloc.tensor_shape)
                dtype = mybir.dt.np(alloc.dtype)
                out_names.append(name)
                out_avals.append(jax.core.ShapedArray(shape, dtype))
                zero_shapes.append((shape, dtype))
        self.in_names, self.out_names = in_names, out_names
        self.out_avals, self.zero_shapes = out_avals, zero_shapes
        n_params, n_outs = len(in_names), len(out_avals)
        all_in = in_names + out_names + ([partition_name] if partition_name else [])

        def _body(*args):
            operands = list(args)
            if partition_name is not None:
                operands.append(partition_id_tensor())
            outs = _bass_exec_p.bind(
                *operands, out_avals=tuple(out_avals), in_names=tuple(all_in),
                out_names=tuple(out_names), lowering_input_output_aliases=(),
                sim_require_finite=True, sim_require_nnan=True, nc=nc)
            return tuple(outs)

        devices = jax.devices()[:n_cores]
        self.mesh = Mesh(np.asarray(devices), ("core",))
        in_specs = (PartitionSpec("core"),) * (n_params + n_outs)
        out_specs = (PartitionSpec("core"),) * n_outs
        self.fn = jax.jit(
            shard_map(_body, mesh=self.mesh, in_specs=in_specs,
                      out_specs=out_specs, check_rep=False),
            keep_unused=True)
        self.sharding = NamedSharding(self.mesh, PartitionSpec("core"))

    def put_inputs(self, in_maps):
        jax = self.jax
        args = []
        for name in self.in_names:
            if name == "dbg_addr" and name not in in_maps[0]:
                per = [np.zeros((1, 2), np.uint32)] * self.n_cores
            else:
                per = [np.asarray(in_maps[c][name]) for c in range(self.n_cores)]
            args.append(jax.device_put(np.concatenate(per, axis=0), self.sharding))
        for shape, dtype in self.zero_shapes:
            z = np.zeros((self.n_cores * shape[0], *shape[1:]), dtype)
            args.append(jax.device_put(z, self.sharding))
        return args

    def run(self, args):
        outs = self.fn(*args)
        self.jax.block_until_ready(outs)
        return outs

    def results(self, outs):
        res = []
        for c in range(self.n_cores):
            d = {}
            for i, name in enumerate(self.out_names):
                shape = self.out_avals[i].shape
                d[name] = np.asarray(outs[i]).reshape(self.n_cores, *shape)[c]
            res.append(d)
        return res


def _get_runner():
    global _RUNNER
    if _RUNNER is None:
        nc = _build_bass()
        _RUNNER = _SpmdRunner(nc, NCORE)
    return _RUNNER


# ================================================================ host-side prep
def _prep_in_maps(feat, feat_a, graph_neigh, W1, att_src, att_dst, W2, Wd1, bd1,
                  gamma, beta, Wd2, bd2, disc_W, disc_b, edge_index):
    feat = np.asarray(feat, np.float32)
    feat_a = np.asarray(feat_a, np.float32)
    gn = np.asarray(graph_neigh, np.float32)
    ei = np.asarray(edge_index).astype(np.int64)
    src, dst = ei[0], ei[1]

    # dense transposed edge-count matrix C^T[s, d]
    CT = np.zeros((N, N), np.float32)
    np.add.at(CT, (src, dst), 1.0)
    assert CT.max() < 256, "bf16 exact-int range exceeded"

    featT = np.ascontiguousarray(feat.T)
    feataT = np.ascontiguousarray(feat_a.T)
    gnT_bf = np.ascontiguousarray(gn.T).astype(BF16)

    att = np.concatenate([np.asarray(att_src, np.float32),
                          np.asarray(att_dst, np.float32)])
    common = {
        "W1": np.asarray(W1, np.float32), "att": att,
        "W2": np.asarray(W2, np.float32), "Wd1": np.asarray(Wd1, np.float32),
        "bd1": np.asarray(bd1, np.float32), "gamma": np.asarray(gamma, np.float32),
        "beta": np.asarray(beta, np.float32), "Wd2bf": np.asarray(Wd2, np.float32).astype(BF16),
        "bd2": np.asarray(bd2, np.float32), "discW": np.asarray(disc_W, np.float32),
        "discb": np.asarray(disc_b, np.float32).reshape(1),
    }
    in_maps = []
    for r in range(NCORE):
        sl = slice(r * NLOC, (r + 1) * NLOC)
        m = dict(common)
        m["featT0"] = np.ascontiguousarray(featT[:, sl])
        m["featT1"] = np.ascontiguousarray(feataT[:, sl])
        m["ct_s"] = np.ascontiguousarray(CT[:, sl]).astype(BF16)
        m["gnT_s"] = np.ascontiguousarray(gnT_bf[:, sl])
        in_maps.append(m)
    return in_maps


def kernel(**inputs):
    runner = _get_runner()
    in_maps = _prep_in_maps(**inputs)
    args = runner.put_inputs(in_maps)
    outs = runner.run(args)
    res = runner.results(outs)
    H2 = np.concatenate([res[r]["out_h2"] for r in range(NCORE)], axis=0)
    H2A = np.concatenate([res[r]["out_h2a"] for r in range(NCORE)], axis=0)
    H3 = np.concatenate([res[r]["out_h3"] for r in range(NCORE)], axis=0)
    RET = np.concatenate([res[r]["out_ret"] for r in range(NCORE)], axis=0)
    RETA = np.concatenate([res[r]["out_reta"] for r in range(NCORE)], axis=0)
    return (H2, H3, RET, RETA, H2, H2A)
